# revision 18
# baseline (speedup 1.0000x reference)
"""Multi-head attention (B=2, S=2048, E=1024, H=16, D=64) on 8 TRN2 cores.

Sharding: core c = b*4 + g  →  batch b ∈ {0,1}, head-group g ∈ {0..3}
(4 heads = 256 embed columns per group).  Each core computes its group's
Q/K/V projections, attention, and the partial output projection
(out^T [1024, 2048], the Wo[:, group]-contracted context).  Host sums the
4 group partials per batch, transposes, and adds bo.

Layout (matmul inputs bf16, all accumulation fp32 in PSUM):
- x is passed pre-transposed per batch: xT [1024, 2048] (e on partitions),
  held in ONE SBUF tile [128, 8, 2048] (e on the free axis) so head DMAs
  can be sliced small and issued across all five sequencer queues.
- Qᵀ/Kᵀ are head-dim-major [256, 2048]: head h of the group lives at
  partition rows (h%2)*64 of tile h//2.  Kᵀ is kept in two zero-padded
  parity copies (ktz/kto) so the scores matmul uses K=128 stationary
  operands (K=64 disables fast weight load).
- V is token-major per 128-token tile as [128, 4*65]: head h at cols
  65h..65h+63, col 65h+64 = 1.0 (ones column → softmax denominator).
- scores are computed transposed (k on partitions, q free) into a single
  6-bank PSUM ring [128, 6, 512] (3 slots x 1 k-tile x 1024 q).  The exp
  stream on ACT reads PAIRS of address-adjacent slots in one instruction
  ([128, 4, 512] = 2048 elem/lane) where the ring phase allows, halving
  the per-instruction ACT overhead (~250ns) vs per-k-tile activations.
  Softmax has no max-subtraction (scores ∈ ±2.8 for this distribution).
- exp output lands in half-combo SBUF tiles [128, 16, 512] (5-deep ring).
- attn·V: lhsT = expᵀ chunk [k,128q], rhs = [V|1] [k,65] → PSUM [128q,65]
  (2-bank pool, alternating); col 64 = denominator → reciprocal +
  tensor_scalar_mul normalization on DVE.
- context (q-major) is PE-transposed in 128x128 blocks; the transposes,
  the out-projection chains and the late Q projections all BORROW idle
  ring banks (no dedicated PSUM pools) — PSUM budget: ring 6 + attnV 2.
- the attn-V/scores pipeline runs combo w's attn-V against combo (w+1)'s
  scores so ACT (the bottleneck engine) never idles mid-stream; combo 1's
  second score-half streams against combo 0's first attn-V chains.
- tail: the last out-projection chunk's DMAs are split [128,256] across
  three queues so the final DMA drain is short, and its PSUM→bf16 copies
  run on ACT (idle after the last exp).
"""

import sys

import numpy as np

_REPO = "/opt/trn_rl_repo"
if _REPO not in sys.path:
    sys.path.insert(0, _REPO)

B, S, E = 2, 2048, 1024
HEADS, D = 16, 64
GROUPS = 4            # head groups (one per core within a batch)
HG = HEADS // GROUPS  # 4 heads per group
FG = HG * D           # 256 embed columns per group
SCALE = D ** -0.5     # 0.125

PF = 128              # partition tile
QC = 512              # free-dim chunk per matmul / PSUM bank (f32)
NE = E // PF          # 8 contraction chunks over embed
NQ = S // QC          # 4 q chunks
NK = S // PF          # 16 k tiles
NS = S // PF          # 16 token tiles
NF = E // PF          # 8 output-feature tiles

_NC_CACHE = None


def _build_nc():
    """Build (once) the single-core Bass/Tile program run SPMD on all 8 cores."""
    global _NC_CACHE
    if _NC_CACHE is not None:
        return _NC_CACHE

    import concourse.bass as bass
    import concourse.tile as tile
    from concourse import bacc, mybir
    from concourse.masks import make_identity

    f32 = mybir.dt.float32
    bf16 = mybir.dt.bfloat16
    Exp = mybir.ActivationFunctionType.Exp
    Copy = mybir.ActivationFunctionType.Copy
    ts = bass.ts

    nc = bacc.Bacc("TRN2", target_bir_lowering=False, debug=False)

    xT_d = nc.declare_dram_parameter("xT", [E, S], bf16, isOutput=False)
    wqT_d = nc.declare_dram_parameter("wqT", [E, FG], bf16, isOutput=False)
    wkT_d = nc.declare_dram_parameter("wkT", [E, FG], bf16, isOutput=False)
    wvT_d = nc.declare_dram_parameter("wvT", [E, FG], bf16, isOutput=False)
    woT_d = nc.declare_dram_parameter("woT", [FG, E], bf16, isOutput=False)
    bq_d = nc.declare_dram_parameter("bq2", [PF, 2], f32, isOutput=False)
    bk_d = nc.declare_dram_parameter("bk2", [PF, 2], f32, isOutput=False)
    bv_d = nc.declare_dram_parameter("bv1", [1, FG], f32, isOutput=False)
    outT_d = nc.declare_dram_parameter("outT", [E, S], bf16, isOutput=True)

    from contextlib import ExitStack

    _stack = ExitStack()
    stack_enter = _stack.enter_context
    with tile.TileContext(nc) as tc:
        with (
            tc.tile_pool(name="w", bufs=1) as pw,
            tc.tile_pool(name="qk", bufs=1) as pqk,
            tc.tile_pool(name="vpool", bufs=1) as pv,
            tc.tile_pool(name="ctx", bufs=1) as pctx,
            tc.tile_pool(name="xt", bufs=1) as px,
            tc.tile_pool(name="pssA", bufs=1, space="PSUM") as pssA,
            tc.tile_pool(name="pssB", bufs=1, space="PSUM") as pssB,
        ):
            # ---- resident weights / biases -------------------------------
            wq_sb = pw.tile([PF, NE, FG], bf16, tag="wq")
            wk_sb = pw.tile([PF, NE, FG], bf16, tag="wk")
            wv_sb = pw.tile([PF, NE, FG], bf16, tag="wv")
            wo_sb = pw.tile([PF, FG // PF, E], bf16, tag="wo")
            bq_sb = pw.tile([PF, 2], f32, tag="bq")
            bk_sb = pw.tile([PF, 2], f32, tag="bk")
            bv_row = pw.tile([1, FG], f32, tag="bvr")
            bv_sb = pw.tile([PF, FG], f32, tag="bvf")
            ident = pw.tile([PF, PF], bf16, tag="ident")
            x_sb = px.tile([PF, NE, S], bf16, tag="x")

            # ---- persistent activations ----------------------------------
            qt_sb = [pqk.tile([PF, S], bf16, tag=f"qt{t}", name=f"qt{t}") for t in range(2)]
            ktz_sb = [pqk.tile([PF, S], bf16, tag=f"ktz{t}", name=f"ktz{t}") for t in range(2)]
            kto_sb = [pqk.tile([PF, S], bf16, tag=f"kto{t}", name=f"kto{t}") for t in range(2)]
            v_sb = [pv.tile([PF, HG * (D + 1)], bf16, tag=f"v{st}", name=f"v{st}") for st in range(NS)]
            ctx_sb = [pctx.tile([PF, FG], bf16, tag=f"ctx{qt}", name=f"ctx{qt}") for qt in range(NS)]
            ctxT_sb = [pctx.tile([PF, S], bf16, tag=f"ctxT{j}", name=f"ctxT{j}") for j in range(2)]

            # ---- head DMA schedule ---------------------------------------
            # per-queue ordered issue lists; criticals (x c0 halves + wk)
            # first so the K(t0,c0) accumulation chain starts asap.  One
            # dma_start runs on ONE dma engine (~20 GB/s), so criticals are
            # split to <=64KB pieces; descriptor generation (~0.6us/issue)
            # runs in parallel across the five sequencer queues.
            def d_x(eng, e, c0, c1):
                eng.dma_start(x_sb[:, e, c0:c1], xT_d[ts(e, PF), c0:c1])

            def d_w(eng, w_sb, w_d, e):
                eng.dma_start(w_sb[:, e, :], w_d[ts(e, PF), :])

            qS, qA, qG = nc.sync, nc.scalar, nc.gpsimd
            # the Kᵀ pad zeros run on DVE (idle until the first bias-add)
            # so they don't clog the gpsimd DMA queue
            for t in range(2):
                nc.vector.memset(ktz_sb[t][D:PF, :], 0.0)
                nc.vector.memset(kto_sb[t][0:D, :], 0.0)
            # round 1 — criticals striped so arrival order matches the
            # K(t0,c0) chain's e order: per e, the two x-c0 halves and wk[e]
            # go to three different queues
            r1 = {qS: [], qA: [], qG: []}
            stripe = [(qS, qA, qG), (qA, qG, qS), (qG, qS, qA)]
            for e in range(NE):
                q1, q2, q3 = stripe[e % 3]
                r1[q1].append(lambda q=q1, e=e: d_x(q, e, 0, 256))
                r1[q2].append(lambda q=q2, e=e: d_x(q, e, 256, 512))
                r1[q3].append(lambda q=q3, e=e: d_w(q, wk_sb, wkT_d, e))
            for q in (qS, qA, qG):
                for fn in r1[q]:
                    fn()
            # round 2 — wq (S/A), biases, then x by consumption deadline:
            # c2 (K/V chains ~20us), c1 small pieces (Q t0 c1 ~17us), c3
            for q, es in ((qS, (0, 2, 4, 6)), (qA, (1, 3, 5, 7))):
                for e in es:
                    d_w(q, wq_sb, wqT_d, e)
            qG.dma_start(bq_sb[:], bq_d[:])
            qG.dma_start(bk_sb[:], bk_d[:])
            qG.dma_start(bv_row[:], bv_d[:])
            for q, es in ((qS, (0, 3, 6)), (qA, (1, 4, 7)), (qG, (2, 5))):
                for e in es:
                    d_x(q, e, 2 * QC, 3 * QC)
            for q, es in ((qS, (0, 2)), (qA, (1, 3)), (qG, (4, 5, 6, 7))):
                for e in es:
                    d_x(q, e, 512, 768)
                    d_x(q, e, 768, 1024)
            nc.gpsimd.partition_broadcast(bv_sb[:], bv_row[:])
            for q, es in ((qS, (0, 3, 6)), (qA, (1, 4, 7)), (qG, (2, 5))):
                for e in es:
                    d_x(q, e, 3 * QC, 4 * QC)
            # round 3 — wv, wo
            for q, es in ((qS, (0, 3, 6)), (qA, (1, 4)), (qG, (2, 5, 7))):
                for e in es:
                    d_w(q, wv_sb, wvT_d, e)
            make_identity(nc, ident[:])
            for e in range(FG // PF):
                qS.dma_start(wo_sb[:, e, :], woT_d[ts(e, PF), :])

            # ---- pools for phase-local state -----------------------------
            pe = stack_enter(tc.tile_pool(name="et", bufs=5))
            pn = stack_enter(tc.tile_pool(name="nrm", bufs=4))
            po_sb = stack_enter(tc.tile_pool(name="owork", bufs=4))
            _p1 = ExitStack()
            _p1.__enter__()
            pps1 = _p1.enter_context(tc.tile_pool(name="ps1", bufs=2, space="PSUM"))

            # ---- scores A/B double-buffer machinery ----------------------
            # k-tiles cycle (A, A, B): the 4-bank A tile takes k-tile pairs
            # (one 2048-elem ACTIVATE), the 2-bank B tile takes singles.
            # While the A act reads, B's matmuls fill, and vice versa, so
            # the exp stream runs back-to-back with ~31% fewer instructions.
            COMBOS = [(0, 0), (0, 1), (0, 2), (0, 3), (1, 0), (1, 1), (1, 2), (1, 3)]
            et_tiles = {}  # (ci, kt//8) -> SBUF [PF, 16, QC] exp half-tile
            state = {"A": None}

            def emit_scores_kt(ci, kt, split_exp=False):
                half, h = COMBOS[ci]
                t = h // 2
                k_sb = ktz_sb[t] if h % 2 == 0 else kto_sb[t]
                key = (ci, kt // 8)
                if key not in et_tiles:
                    et_tiles[key] = pe.tile(
                        [PF, NK, QC], bf16, tag="et", name=f"et{ci}_{kt // 8}"
                    )
                et = et_tiles[key]
                pos = kt % 3
                if pos == 2:
                    ps = pssB.tile([PF, 2, QC], f32, tag="pssB", name="pssB")
                    base = 0
                else:
                    if pos == 0:
                        state["A"] = pssA.tile([PF, 4, QC], f32, tag="pssA", name="pssA")
                    ps = state["A"]
                    base = 2 * pos
                lk = 2 * (kt % 8)
                for j in range(2):
                    nc.tensor.matmul(
                        ps[:, base + j, :],
                        k_sb[:, ts(kt, PF)],
                        qt_sb[t][:, ts(half * 2 + j, QC)],
                        start=True,
                        stop=True,
                    )
                    if split_exp:
                        nc.scalar.activation(
                            et[:, lk + j, :], ps[:, base + j, :], Exp, scale=SCALE
                        )
                if split_exp:
                    return
                if pos == 1:  # pair act over the full A tile (k-tiles kt-1, kt)
                    nc.scalar.activation(
                        et[:, lk - 2 : lk + 2, :], ps[:, 0:4, :], Exp, scale=SCALE
                    )
                elif pos == 2 or kt == NK - 1:  # B single / combo-final single
                    nc.scalar.activation(
                        et[:, lk : lk + 2, :], ps[:, base : base + 2, :], Exp, scale=SCALE
                    )

            # ---- phase-1 projections -------------------------------------
            def project_qk(w_sb, b_sb, o_sb, t, c):
                ps = pps1.tile([PF, QC], f32, tag="ps1", name="ps1")
                for e in range(NE):
                    nc.tensor.matmul(
                        ps[:],
                        w_sb[:, e, ts(t, PF)],
                        x_sb[:, e, ts(c, QC)],
                        start=(e == 0),
                        stop=(e == NE - 1),
                    )
                if o_sb is None:  # K: split into the parity copies
                    nc.vector.tensor_scalar_add(
                        ktz_sb[t][0:D, ts(c, QC)], ps[0:D, :], b_sb[0:D, t : t + 1]
                    )
                    nc.vector.tensor_scalar_add(
                        kto_sb[t][D:PF, ts(c, QC)], ps[D:PF, :], b_sb[D:PF, t : t + 1]
                    )
                else:
                    nc.vector.tensor_scalar_add(
                        o_sb[t][:, ts(c, QC)], ps[:], b_sb[:, t : t + 1]
                    )

            def emit_v(st):
                ps = pps1.tile([PF, FG], f32, tag="ps1", name="ps1v")
                for e in range(NE):
                    nc.tensor.matmul(
                        ps[:],
                        x_sb[:, e, ts(st, PF)],
                        wv_sb[:, e, :],
                        start=(e == 0),
                        stop=(e == NE - 1),
                    )
                nc.vector.memset(v_sb[st][:], 1.0)
                for h in range(HG):
                    nc.vector.tensor_add(
                        v_sb[st][:, h * (D + 1) : h * (D + 1) + D],
                        ps[:, ts(h, D)],
                        bv_sb[:, ts(h, D)],
                    )

            def project_q_late(t, c):
                ps = ppso.tile([PF, QC], f32, tag="pso", name="psoq")
                for e in range(NE):
                    nc.tensor.matmul(
                        ps,
                        wq_sb[:, e, ts(t, PF)],
                        x_sb[:, e, ts(c, QC)],
                        start=(e == 0),
                        stop=(e == NE - 1),
                    )
                nc.vector.tensor_scalar_add(
                    qt_sb[t][:, ts(c, QC)], ps[:], bq_sb[:, t : t + 1]
                )

            # ---- attention -----------------------------------------------
            def emit_attnv_qs(ci, qs):
                half, h = COMBOS[ci]
                j, sub = qs // 4, qs % 4
                po = ppso.tile([PF, D + 1], f32, tag="pso", name="pso")
                for kt in range(NK):
                    et = et_tiles[(ci, kt // 8)]
                    nc.tensor.matmul(
                        po[:],
                        et[:, 2 * (kt % 8) + j, ts(sub, PF)],
                        v_sb[kt][:, h * (D + 1) : (h + 1) * (D + 1)],
                        start=(kt == 0),
                        stop=(kt == NK - 1),
                    )
                qt = half * 8 + qs
                r = pn.tile([PF, 1], f32, tag="r", name="r")
                nc.vector.reciprocal(r[:], po[:, D : D + 1])
                nc.vector.tensor_scalar_mul(
                    ctx_sb[qt][:, ts(h, D)], po[:, 0:D], r[:]
                )

            def emit_transpose_qt(qt):
                for j2 in range(2):
                    pap = ppso.tile([PF, PF], bf16, tag="pso", name="ptr")
                    nc.tensor.transpose(
                        pap[:], ctx_sb[qt][:, ts(j2, PF)], ident[:]
                    )
                    nc.vector.tensor_copy(ctxT_sb[j2][:, ts(qt, PF)], pap[:])

            oidx = [0]

            def emit_outproj_group(c, ft, tail=False):
                ps = ppso.tile([PF, QC], f32, tag="pso", name="psoo")
                for e in range(FG // PF):
                    nc.tensor.matmul(
                        ps[:],
                        wo_sb[:, e, ts(ft, PF)],
                        ctxT_sb[e][:, ts(c, QC)],
                        start=(e == 0),
                        stop=(e == FG // PF - 1),
                    )
                ot = po_sb.tile([PF, QC], bf16, tag="ot", name="ot")
                if tail:
                    nc.scalar.activation(ot[:], ps[:], Copy)
                    for hi in range(2):
                        eng = (qS, qG, qA)[oidx[0] % 3]
                        oidx[0] += 1
                        eng.dma_start(
                            outT_d[ts(ft, PF), c * QC + hi * 256 : c * QC + (hi + 1) * 256],
                            ot[:, hi * 256 : (hi + 1) * 256],
                        )
                else:
                    nc.vector.tensor_copy(ot[:], ps[:])
                    oidx[0] += 1
                    eng = qS if oidx[0] % 2 == 0 else qG
                    eng.dma_start(outT_d[ts(ft, PF), ts(c, QC)], ot[:])

            # ---- phase 1: projections + combos 0 / 1a --------------------
            # interleave K-tile-0 projections with combo (0,0)'s scores so
            # the exp stream (ACT, the bottleneck) starts asap; V-projection
            # groups spread through the chain so all of V is done by the
            # time the first attn-V group runs
            project_qk(wk_sb, bk_sb, None, 0, 0)
            project_qk(wq_sb, bq_sb, qt_sb, 0, 0)
            project_qk(wq_sb, bq_sb, qt_sb, 0, 1)
            for kt in range(4):
                emit_scores_kt(0, kt, split_exp=True)
            for c in range(1, NQ):
                project_qk(wk_sb, bk_sb, None, 0, c)
                for st in range(4 * (c - 1), 4 * c):
                    emit_v(st)
                for kt in range(4 * c, 4 * c + 4):
                    emit_scores_kt(0, kt, split_exp=True)
            # combo 1's first half streams against the phase-1 tail
            kt01 = [0]
            tail_work = (
                [lambda st=st: emit_v(st) for st in range(12, 16)]
                + [lambda c=c: project_qk(wk_sb, bk_sb, None, 1, c) for c in range(NQ)]
                + [lambda c=c: project_qk(wq_sb, bq_sb, qt_sb, 1, c) for c in range(2)]
            )
            for fn in tail_work:
                fn()
                if kt01[0] < 8:
                    emit_scores_kt(1, kt01[0])
                    kt01[0] += 1
            while kt01[0] < 8:
                emit_scores_kt(1, kt01[0])
                kt01[0] += 1

            # ---- phase 2: attention --------------------------------------
            _p1.close()
            ppso = stack_enter(tc.tile_pool(name="pso", bufs=2, space="PSUM"))

            pending = [
                lambda t=t, c=c: project_q_late(t, c)
                for t in range(2)
                for c in range(2, NQ)
            ]

            # window 0: combo 1's second half against combo 0's first chains.
            # ring-borrowing pops run BEFORE an even k-tile's scores: after
            # an odd k-tile the pending pair is always flushed, so the
            # borrow slot (G+2)%3 ≡ (G-1)%3 holds already-acted scores.
            for kt in range(8, NK):
                if kt % 2 == 0 and pending:
                    pending.pop(0)()
                emit_scores_kt(1, kt)
                if kt % 2 == 1:
                    emit_attnv_qs(0, (kt - 8) // 2)

            # windows 1..7: combo w's attn-V against combo (w+1)'s scores.
            # windows 1-2 carry 12 chains each (combo 0's tail + their own).
            extra_chains = [(0, qs) for qs in range(4, 8)]
            extras = {
                3: [
                    lambda: pending.extend(
                        (lambda qt=qt: emit_transpose_qt(qt)) for qt in range(8)
                    )
                ],
                4: [
                    lambda: pending.extend(
                        (lambda ft=ft: emit_outproj_group(0, ft)) for ft in range(NF)
                    )
                ],
                5: [
                    lambda: pending.extend(
                        (lambda ft=ft: emit_outproj_group(1, ft)) for ft in range(NF)
                    )
                ],
            }
            for av in range(1, 8):
                cur = av + 1 if av + 1 < 8 else None
                for kt in range(NK):
                    if kt % 2 == 0:
                        if extra_chains:
                            # combo 0's tail chains must clear before the
                            # et ring slot they read is re-acted (kt 8)
                            ci, qs = extra_chains.pop(0)
                            emit_attnv_qs(ci, qs)
                        elif pending:
                            # pop BEFORE this k-tile's scores so the borrow
                            # slot holds already-acted scores (see window 0)
                            pending.pop(0)()
                    if cur is not None:
                        emit_scores_kt(cur, kt)
                    if kt % 2 == 1:
                        emit_attnv_qs(av, kt // 2)
                        # tail: peel transposes/out-proj in as soon as
                        # their context tiles complete
                        if av == 7:
                            if kt == 7:
                                for qt in range(8, 12):
                                    emit_transpose_qt(qt)
                                pending.extend(
                                    (lambda ft=ft: emit_outproj_group(2, ft))
                                    for ft in range(NF)
                                )
                            elif kt > 8:
                                emit_transpose_qt(12 + kt // 2 - 4)
                for fn in extras.get(av, ()):
                    fn()
            while pending:
                pending.pop(0)()
            for ft in range(NF):
                emit_outproj_group(3, ft, tail=True)

            _stack.close()

    nc.compile()
    _NC_CACHE = nc
    return nc


def _in_maps(x, Wq, bq, Wk, bk, Wv, bv, Wo, bo):
    """Per-core input dicts: core c = b*4 + g."""
    import ml_dtypes

    f = np.float32
    b16 = ml_dtypes.bfloat16
    maps = []
    for b in range(B):
        xT = np.ascontiguousarray(x[b].T).astype(b16)
        for g in range(GROUPS):
            gs = g * FG
            sl = slice(gs, gs + FG)
            maps.append(
                {
                    "xT": xT,
                    "wqT": np.ascontiguousarray(Wq[sl, :].T).astype(b16),
                    "wkT": np.ascontiguousarray(Wk[sl, :].T).astype(b16),
                    "wvT": np.ascontiguousarray(Wv[sl, :].T).astype(b16),
                    "woT": np.ascontiguousarray(Wo[:, sl].T).astype(b16),
                    "bq2": np.ascontiguousarray(bq[sl].reshape(2, PF).T, dtype=f),
                    "bk2": np.ascontiguousarray(bk[sl].reshape(2, PF).T, dtype=f),
                    "bv1": np.ascontiguousarray(bv[sl].reshape(1, FG), dtype=f),
                }
            )
    return maps


def _assemble(results, bo):
    out = np.empty((B, S, E), dtype=np.float32)
    for b in range(B):
        acc = results[b * GROUPS]["outT"].astype(np.float32, copy=True)
        for g in range(1, GROUPS):
            acc += results[b * GROUPS + g]["outT"]
        out[b] = acc.T + bo.astype(np.float32)
    return out


def kernel(x, Wq, bq, Wk, bk, Wv, bv, Wo, bo):
    from concourse.bass_utils import run_bass_kernel_spmd

    nc = _build_nc()
    maps = _in_maps(x, Wq, bq, Wk, bk, Wv, bv, Wo, bo)
    res = run_bass_kernel_spmd(nc, maps, core_ids=list(range(8)))
    return _assemble(res.results, np.asarray(bo))
